# revision 4
# baseline (speedup 1.0000x reference)
"""L-infinity distance "convolution" kernel for Trainium2 (8 NeuronCores).

Computes out[b, co, h, w] = max_acc |weights[co, acc] - patch[b, h, w, acc]| + bias[co]
where patches are 3x3 replicate-padded windows over x (4, 16, 64, 64),
acc = (c, kh, kw) ordered, accl = 16*9 = 144, cout = 64.

Sharding: 8 cores = 4 batches x 2 row-halves. Each core computes a
[2048 positions, 64 cout] shard. No collectives needed.

Device layout: partitions = 128 spatial positions per tile (16 tiles/core),
free dim = (cout, acc). Weights are broadcast-replicated across partitions
once; per tile one big tensor_tensor subtract + one segmented
tensor_reduce(max, abs) + bias add.
"""

import numpy as np

B, C, H, W = 4, 16, 64, 64
K = 3
COUT = 64
ACC = C * K * K  # 144
HOUT, WOUT = 64, 64
NPOS = HOUT * WOUT  # 4096
NCORES = 8
HALVES = 2
POS_PER_CORE = NPOS // HALVES  # 2048
P = 128  # partitions
NTILES = POS_PER_CORE // P  # 16

_TRACE = False


def _build_bass():
    import concourse.bass as bass
    import concourse.bacc as bacc
    import concourse.mybir as mybir
    import concourse.tile as tile
    from concourse.alu_op_type import AluOpType

    nc = bacc.Bacc("TRN2", target_bir_lowering=False, debug=False, num_devices=NCORES)
    patches_d = nc.dram_tensor(
        "patches", [POS_PER_CORE, ACC], mybir.dt.float32, kind="ExternalInput"
    )
    w_d = nc.dram_tensor(
        "w", [1, COUT * ACC], mybir.dt.float32, kind="ExternalInput"
    )
    bias_d = nc.dram_tensor("bias", [1, COUT], mybir.dt.float32, kind="ExternalInput")
    out_d = nc.dram_tensor(
        "out", [POS_PER_CORE, COUT], mybir.dt.float32, kind="ExternalOutput"
    )

    with tile.TileContext(nc) as tc:
        with (
            tc.tile_pool(name="consts", bufs=1) as consts,
            tc.tile_pool(name="work", bufs=3) as work,
            tc.tile_pool(name="dwork", bufs=2) as dwork,
            tc.tile_pool(name="outp", bufs=3) as outp,
        ):
            w_rep = consts.tile([P, COUT * ACC], mybir.dt.float32)
            bias_rep = consts.tile([P, COUT], mybir.dt.float32)

            # Broadcast weights across all 128 partitions via a
            # partition-step-0 DRAM-side AP. Single DMA so downstream
            # consumers only need one sync wait.
            base = w_d[0:1, :]
            src = bass.AP(
                tensor=base.tensor, offset=base.offset, ap=[[0, P], [1, COUT * ACC]]
            )
            nc.sync.dma_start(out=w_rep[:, :], in_=src)
            bbase = bias_d[0:1, :]
            bsrc = bass.AP(
                tensor=bbase.tensor, offset=bbase.offset, ap=[[0, P], [1, COUT]]
            )
            nc.sync.dma_start(out=bias_rep[:, :], in_=bsrc)

            w3 = w_rep[:, :].rearrange("p (c a) -> p c a", a=ACC)

            for t in range(NTILES):
                pt = work.tile([P, ACC], mybir.dt.float32)
                nc.sync.dma_start(
                    out=pt[:, :], in_=patches_d[t * P : (t + 1) * P, :]
                )
                d = dwork.tile([P, COUT * ACC], mybir.dt.float32)
                d3 = d[:, :].rearrange("p (c a) -> p c a", a=ACC)
                pt_b = pt[:, :].unsqueeze(1).broadcast_to([P, COUT, ACC])
                nc.vector.tensor_tensor(
                    out=d3, in0=w3, in1=pt_b, op=AluOpType.subtract
                )
                dist = outp.tile([P, COUT], mybir.dt.float32)
                nc.vector.tensor_reduce(
                    out=dist[:, :],
                    in_=d3,
                    axis=mybir.AxisListType.X,
                    op=AluOpType.max,
                    apply_absolute_value=True,
                )
                nc.vector.tensor_tensor(
                    out=dist[:, :], in0=dist[:, :], in1=bias_rep[:, :], op=AluOpType.add
                )
                nc.sync.dma_start(
                    out=out_d[t * P : (t + 1) * P, :], in_=dist[:, :]
                )
    nc.compile()
    return nc


def _host_prep(inputs):
    x = np.asarray(inputs["x"], dtype=np.float32)
    weights = np.asarray(inputs["weights"], dtype=np.float32)
    bias = np.asarray(inputs["bias"], dtype=np.float32)
    assert x.shape == (B, C, H, W)
    assert weights.shape == (COUT, ACC)

    x_pad = np.pad(x, ((0, 0), (0, 0), (1, 1), (1, 1)), mode="edge")
    from numpy.lib.stride_tricks import sliding_window_view

    pw = sliding_window_view(x_pad, (K, K), axis=(2, 3))  # (B, C, HOUT, WOUT, K, K)
    patches = np.ascontiguousarray(pw.transpose(0, 2, 3, 1, 4, 5)).reshape(
        B, NPOS, ACC
    )
    wflat = np.ascontiguousarray(weights.reshape(1, COUT * ACC))
    bflat = np.ascontiguousarray(bias.reshape(1, COUT))
    return patches, wflat, bflat


_NC_CACHE = None


def _get_nc():
    global _NC_CACHE
    if _NC_CACHE is None:
        _NC_CACHE = _build_bass()
    return _NC_CACHE


def _run(inputs, trace=False):
    from concourse.bass_utils import run_bass_kernel_spmd

    patches, wflat, bflat = _host_prep(inputs)
    in_maps = []
    for core in range(NCORES):
        b, half = core // HALVES, core % HALVES
        shard = np.ascontiguousarray(
            patches[b, half * POS_PER_CORE : (half + 1) * POS_PER_CORE, :]
        )
        in_maps.append({"patches": shard, "w": wflat, "bias": bflat})

    nc = _get_nc()
    res = run_bass_kernel_spmd(
        nc, in_maps, core_ids=list(range(NCORES)), trace=trace
    )

    rows_per_half = POS_PER_CORE // WOUT  # 32
    out = np.empty((B, COUT, HOUT, WOUT), dtype=np.float32)
    for core in range(NCORES):
        b, half = core // HALVES, core % HALVES
        o = res.results[core]["out"]  # [POS_PER_CORE, COUT]
        out[b, :, half * rows_per_half : (half + 1) * rows_per_half, :] = (
            o.T.reshape(COUT, rows_per_half, WOUT)
        )
    return out, res


def kernel(**inputs) -> np.ndarray:
    out, _ = _run(inputs, trace=_TRACE)
    return out


# revision 5
# speedup vs baseline: 1.3998x; 1.3998x over previous
"""L-infinity distance "convolution" kernel for Trainium2 (8 NeuronCores).

Computes out[b, co, h, w] = max_acc |weights[co, acc] - patch[b, h, w, acc]| + bias[co]
where patches are 3x3 replicate-padded windows over x (4, 16, 64, 64),
acc = (c, kh, kw) ordered, accl = 16*9 = 144, cout = 64.

Sharding: 8 cores = 4 batches x 2 row-halves. Each core computes a
[2048 positions, 64 cout] shard. No collectives needed.

Device layout: partitions = 128 spatial positions per tile (16 tiles/core),
weights replicated across partitions (one broadcast DMA per 8-cout chunk).
Inner loop: one custom DVE instruction per (tile, cout) -- a fused
scan-max over |w - patch| whose output AP has step 0, so the last scan
element (the full 144-way reduction) lands directly in dist[:, co].
"""

import numpy as np

B, C, H, W = 4, 16, 64, 64
K = 3
COUT = 64
ACC = C * K * K  # 144
HOUT, WOUT = 64, 64
NPOS = HOUT * WOUT  # 4096
NCORES = 8
HALVES = 2
POS_PER_CORE = NPOS // HALVES  # 2048
P = 128  # partitions
NTILES = POS_PER_CORE // P  # 16
COG = 8  # cout chunk for weight broadcast tiles
NCHUNK = COUT // COG  # 8

_TRACE = False

_OP_CACHE = None


def _get_op():
    """Register (once) the fused |a-b| scan-max custom DVE op."""
    global _OP_CACHE
    if _OP_CACHE is not None:
        return _OP_CACHE
    from concourse.dve_spec import Spec, Src0, Src1, maxx, lower, AluOp, _has_src1, scan
    from concourse.dve_uop import DveOpSpec
    import concourse.dve_ops as dve_ops
    from concourse.dve_ops import DveOp

    def _ref(in0, in1, s0, s1, imm2):
        b = np.maximum.accumulate(
            np.abs(in0.astype(np.float32) - in1), axis=-1
        )
        return b.astype(np.float32)

    spec = Spec(
        body=scan(AluOp.MAX, maxx(Src0 - Src1, Src1 - Src0)), reference=_ref
    )
    name = "ABSDIFF_MAX_SCAN"
    if name not in dve_ops._SUB_OPCODE_FOR_NAME:
        row = max(dve_ops._SUB_OPCODE_FOR_NAME.values()) + 1
        assert row < 0x20
        dve_ops._SUB_OPCODE_FOR_NAME[name] = row
    row = dve_ops._SUB_OPCODE_FOR_NAME[name]
    shas = {}
    for ver in ("v3", "v4"):
        s = DveOpSpec(
            name=name, opcode=row, uops=lower(spec, ver=ver), rd1_en=_has_src1(spec)
        )
        shas[ver] = s.sha(ver)
    op = DveOp(name, spec, subdim=False, uops_sha=shas)
    if all(o.name != name for o in dve_ops.OPS):
        dve_ops.OPS.append(op)
        dve_ops.CUSTOM_DVE_SPECS[name] = spec
    _OP_CACHE = op
    return op


def _build_bass():
    import concourse.bass as bass
    import concourse.bacc as bacc
    import concourse.mybir as mybir
    import concourse.tile as tile
    from concourse.alu_op_type import AluOpType

    op = _get_op()

    nc = bacc.Bacc("TRN2", target_bir_lowering=False, debug=False, num_devices=NCORES)
    patches_d = nc.dram_tensor(
        "patches", [POS_PER_CORE, ACC], mybir.dt.float32, kind="ExternalInput"
    )
    w_d = nc.dram_tensor("w", [1, COUT * ACC], mybir.dt.float32, kind="ExternalInput")
    bias_d = nc.dram_tensor("bias", [1, COUT], mybir.dt.float32, kind="ExternalInput")
    out_d = nc.dram_tensor(
        "out", [POS_PER_CORE, COUT], mybir.dt.float32, kind="ExternalOutput"
    )

    with tile.TileContext(nc) as tc:
        with (
            tc.tile_pool(name="consts", bufs=1) as consts,
            tc.tile_pool(name="work", bufs=3) as work,
            tc.tile_pool(name="outp", bufs=3) as outp,
        ):
            # weights replicated across partitions, in NCHUNK separate tiles
            # so early compute only waits on its own chunk's DMA
            wchunks = []
            for g in range(NCHUNK):
                wt = consts.tile([P, COG * ACC], mybir.dt.float32, tag=f"wch{g}")
                base = w_d[0:1, g * COG * ACC : (g + 1) * COG * ACC]
                src = bass.AP(
                    tensor=base.tensor, offset=base.offset, ap=[[0, P], [1, COG * ACC]]
                )
                nc.sync.dma_start(out=wt[:, :], in_=src)
                wchunks.append(wt)
            bias_rep = consts.tile([P, COUT], mybir.dt.float32)
            bbase = bias_d[0:1, :]
            bsrc = bass.AP(
                tensor=bbase.tensor, offset=bbase.offset, ap=[[0, P], [1, COUT]]
            )
            nc.sync.dma_start(out=bias_rep[:, :], in_=bsrc)

            for t in range(NTILES):
                pt = work.tile([P, ACC], mybir.dt.float32)
                nc.sync.dma_start(out=pt[:, :], in_=patches_d[t * P : (t + 1) * P, :])
                dist = outp.tile([P, COUT], mybir.dt.float32)
                for co in range(COUT):
                    g, j = co // COG, co % COG
                    w_slice = wchunks[g][:, j * ACC : (j + 1) * ACC]
                    d0 = dist[:, co : co + 1]
                    squash = bass.AP(
                        tensor=d0.tensor, offset=d0.offset, ap=[d0.ap[0], [0, ACC]]
                    )
                    nc.vector._custom_dve(op, out=squash, in0=w_slice, in1=pt[:, :])
                nc.vector.tensor_tensor(
                    out=dist[:, :], in0=dist[:, :], in1=bias_rep[:, :], op=AluOpType.add
                )
                nc.sync.dma_start(out=out_d[t * P : (t + 1) * P, :], in_=dist[:, :])
    nc.compile()
    return nc


def _host_prep(inputs):
    x = np.asarray(inputs["x"], dtype=np.float32)
    weights = np.asarray(inputs["weights"], dtype=np.float32)
    bias = np.asarray(inputs["bias"], dtype=np.float32)
    assert x.shape == (B, C, H, W)
    assert weights.shape == (COUT, ACC)

    x_pad = np.pad(x, ((0, 0), (0, 0), (1, 1), (1, 1)), mode="edge")
    from numpy.lib.stride_tricks import sliding_window_view

    pw = sliding_window_view(x_pad, (K, K), axis=(2, 3))  # (B, C, HOUT, WOUT, K, K)
    patches = np.ascontiguousarray(pw.transpose(0, 2, 3, 1, 4, 5)).reshape(
        B, NPOS, ACC
    )
    wflat = np.ascontiguousarray(weights.reshape(1, COUT * ACC))
    bflat = np.ascontiguousarray(bias.reshape(1, COUT))
    return patches, wflat, bflat


_NC_CACHE = None


def _get_nc():
    global _NC_CACHE
    if _NC_CACHE is None:
        _NC_CACHE = _build_bass()
    return _NC_CACHE


def _run(inputs, trace=False):
    from concourse.bass_utils import run_bass_kernel_spmd

    patches, wflat, bflat = _host_prep(inputs)
    in_maps = []
    for core in range(NCORES):
        b, half = core // HALVES, core % HALVES
        shard = np.ascontiguousarray(
            patches[b, half * POS_PER_CORE : (half + 1) * POS_PER_CORE, :]
        )
        in_maps.append({"patches": shard, "w": wflat, "bias": bflat})

    nc = _get_nc()
    res = run_bass_kernel_spmd(nc, in_maps, core_ids=list(range(NCORES)), trace=trace)

    rows_per_half = POS_PER_CORE // WOUT  # 32
    out = np.empty((B, COUT, HOUT, WOUT), dtype=np.float32)
    for core in range(NCORES):
        b, half = core // HALVES, core % HALVES
        o = res.results[core]["out"]  # [POS_PER_CORE, COUT]
        out[b, :, half * rows_per_half : (half + 1) * rows_per_half, :] = o.T.reshape(
            COUT, rows_per_half, WOUT
        )
    return out, res


def kernel(**inputs) -> np.ndarray:
    out, _ = _run(inputs, trace=_TRACE)
    return out


# revision 10
# speedup vs baseline: 1.4812x; 1.0581x over previous
"""L-infinity distance "convolution" kernel for Trainium2 (8 NeuronCores).

Computes out[b, co, h, w] = max_acc |weights[co, acc] - patch[b, h, w, acc]| + bias[co]
where patches are 3x3 replicate-padded windows over x (4, 16, 64, 64),
acc = (c, kh, kw) ordered, accl = 16*9 = 144, cout = 64.

Sharding: 8 cores = 4 batches x 2 row-halves. Each core computes a
[2048 positions, 64 cout] shard. No collectives needed.

Device layout: partitions = 128 spatial positions per tile (16 tiles/core),
weights replicated across partitions (one broadcast DMA per 8-cout chunk).
Inner loop: one custom DVE instruction per (tile, cout) -- a fused
scan-max over |w - patch| whose output AP has step 0, so the last scan
element (the full 144-way reduction) lands directly in dist[:, co].
"""

import numpy as np

B, C, H, W = 4, 16, 64, 64
K = 3
COUT = 64
ACC = C * K * K  # 144
HOUT, WOUT = 64, 64
NPOS = HOUT * WOUT  # 4096
NCORES = 8
HALVES = 2
POS_PER_CORE = NPOS // HALVES  # 2048
P = 128  # partitions
NTILES = POS_PER_CORE // P  # 16
COG = 8  # cout chunk for weight broadcast tiles
NCHUNK = COUT // COG  # 8
GPS_SPLIT = True  # offload half the tiles' subtract to gpsimd

_TRACE = False

_OP_CACHE = None


def _get_op():
    """Register (once) the fused |a-b| scan-max custom DVE op."""
    global _OP_CACHE
    if _OP_CACHE is not None:
        return _OP_CACHE
    from concourse.dve_spec import (
        Spec,
        Src0,
        Src1,
        C0,
        maxx,
        lower,
        AluOp,
        _has_src1,
        scan,
    )
    from concourse.dve_uop import DveOpSpec
    import concourse.dve_ops as dve_ops
    from concourse.dve_ops import DveOp

    def _ref(in0, in1, s0, s1, imm2):
        b = np.maximum.accumulate(
            np.abs(in0.astype(np.float32) - in1), axis=-1
        )
        return (b + s0).astype(np.float32)

    spec = Spec(
        body=scan(AluOp.MAX, maxx(Src0 - Src1, Src1 - Src0)) + C0, reference=_ref
    )
    name = "ABSDIFF_MAX_SCAN"
    if name not in dve_ops._SUB_OPCODE_FOR_NAME:
        row = max(dve_ops._SUB_OPCODE_FOR_NAME.values()) + 1
        assert row < 0x20
        dve_ops._SUB_OPCODE_FOR_NAME[name] = row
    row = dve_ops._SUB_OPCODE_FOR_NAME[name]
    shas = {}
    for ver in ("v3", "v4"):
        s = DveOpSpec(
            name=name, opcode=row, uops=lower(spec, ver=ver), rd1_en=_has_src1(spec)
        )
        shas[ver] = s.sha(ver)
    op = DveOp(name, spec, subdim=False, uops_sha=shas)
    if all(o.name != name for o in dve_ops.OPS):
        dve_ops.OPS.append(op)
        dve_ops.CUSTOM_DVE_SPECS[name] = spec
    _OP_CACHE = op
    return op


def _build_bass():
    import concourse.bass as bass
    import concourse.bacc as bacc
    import concourse.mybir as mybir
    import concourse.tile as tile
    from concourse.alu_op_type import AluOpType

    op = _get_op()

    nc = bacc.Bacc("TRN2", target_bir_lowering=False, debug=False, num_devices=NCORES)
    patches_d = nc.dram_tensor(
        "patches", [POS_PER_CORE, ACC], mybir.dt.float32, kind="ExternalInput"
    )
    w_d = nc.dram_tensor("w", [1, COUT * ACC], mybir.dt.float32, kind="ExternalInput")
    bias_d = nc.dram_tensor("bias", [1, COUT], mybir.dt.float32, kind="ExternalInput")
    out_d = nc.dram_tensor(
        "out", [POS_PER_CORE, COUT], mybir.dt.float32, kind="ExternalOutput"
    )

    # Tiles handled by gpsimd (big tensor_tensor subtract + DVE segmented
    # reduce) vs DVE-only fused squash ops. Interleaved so both engines
    # stay busy concurrently.
    gps_tiles = set(range(0, NTILES, 2)) if GPS_SPLIT else set()

    with tile.TileContext(nc) as tc:
        with (
            tc.tile_pool(name="consts", bufs=1) as consts,
            tc.tile_pool(name="work", bufs=4) as work,
            tc.tile_pool(name="dwork", bufs=2) as dwork,
            tc.tile_pool(name="outp", bufs=4) as outp,
        ):
            # weights replicated across partitions, in NCHUNK separate tiles
            # so early compute only waits on its own chunk's DMA
            wchunks = []
            for g in range(NCHUNK):
                wt = consts.tile([P, COG * ACC], mybir.dt.float32, tag=f"wch{g}")
                base = w_d[0:1, g * COG * ACC : (g + 1) * COG * ACC]
                src = bass.AP(
                    tensor=base.tensor, offset=base.offset, ap=[[0, P], [1, COG * ACC]]
                )
                nc.sync.dma_start(out=wt[:, :], in_=src)
                wchunks.append(wt)
            bias_rep = consts.tile([P, COUT], mybir.dt.float32)
            bbase = bias_d[0:1, :]
            bsrc = bass.AP(
                tensor=bbase.tensor, offset=bbase.offset, ap=[[0, P], [1, COUT]]
            )
            nc.sync.dma_start(out=bias_rep[:, :], in_=bsrc)

            w_full = None
            if gps_tiles:
                # contiguous view of all weights for the big gpsimd subtract
                w_full = consts.tile([P, COUT * ACC], mybir.dt.float32)
                base = w_d[0:1, :]
                src = bass.AP(
                    tensor=base.tensor, offset=base.offset, ap=[[0, P], [1, COUT * ACC]]
                )
                nc.sync.dma_start(out=w_full[:, :], in_=src)

            pending = []  # gpsimd tiles awaiting their DVE reduce

            def flush_pending():
                while pending:
                    t0, d_t = pending.pop(0)
                    dist = outp.tile([P, COUT], mybir.dt.float32, tag="dist")
                    d3 = d_t[:, :].rearrange("p (c a) -> p c a", a=ACC)
                    nc.vector.tensor_reduce(
                        out=dist[:, :],
                        in_=d3,
                        axis=mybir.AxisListType.X,
                        op=AluOpType.max,
                        apply_absolute_value=True,
                    )
                    nc.vector.tensor_tensor(
                        out=dist[:, :],
                        in0=dist[:, :],
                        in1=bias_rep[:, :],
                        op=AluOpType.add,
                    )
                    nc.sync.dma_start(
                        out=out_d[t0 * P : (t0 + 1) * P, :], in_=dist[:, :]
                    )

            for t in range(NTILES):
                pt = work.tile([P, ACC], mybir.dt.float32, tag="pt")
                nc.sync.dma_start(out=pt[:, :], in_=patches_d[t * P : (t + 1) * P, :])
                if t in gps_tiles:
                    d_t = dwork.tile([P, COUT * ACC], mybir.dt.float32, tag="D")
                    d3 = d_t[:, :].rearrange("p (c a) -> p c a", a=ACC)
                    w3 = w_full[:, :].rearrange("p (c a) -> p c a", a=ACC)
                    pt_b = pt[:, :].unsqueeze(1).broadcast_to([P, COUT, ACC])
                    nc.gpsimd.tensor_tensor(
                        out=d3, in0=w3, in1=pt_b, op=AluOpType.subtract
                    )
                    pending.append((t, d_t))
                else:
                    dist = outp.tile([P, COUT], mybir.dt.float32, tag="dist")
                    for co in range(COUT):
                        g, j = co // COG, co % COG
                        w_slice = wchunks[g][:, j * ACC : (j + 1) * ACC]
                        d0 = dist[:, co : co + 1]
                        squash = bass.AP(
                            tensor=d0.tensor, offset=d0.offset, ap=[d0.ap[0], [0, ACC]]
                        )
                        nc.vector._custom_dve(
                            op,
                            out=squash,
                            in0=w_slice,
                            in1=pt[:, :],
                            s0=bias_rep[:, co : co + 1],
                        )
                    nc.sync.dma_start(
                        out=out_d[t * P : (t + 1) * P, :], in_=dist[:, :]
                    )
                    flush_pending()
            flush_pending()
    nc.compile()
    return nc


def _host_prep(inputs):
    x = np.asarray(inputs["x"], dtype=np.float32)
    weights = np.asarray(inputs["weights"], dtype=np.float32)
    bias = np.asarray(inputs["bias"], dtype=np.float32)
    assert x.shape == (B, C, H, W)
    assert weights.shape == (COUT, ACC)

    x_pad = np.pad(x, ((0, 0), (0, 0), (1, 1), (1, 1)), mode="edge")
    from numpy.lib.stride_tricks import sliding_window_view

    pw = sliding_window_view(x_pad, (K, K), axis=(2, 3))  # (B, C, HOUT, WOUT, K, K)
    patches = np.ascontiguousarray(pw.transpose(0, 2, 3, 1, 4, 5)).reshape(
        B, NPOS, ACC
    )
    wflat = np.ascontiguousarray(weights.reshape(1, COUT * ACC))
    bflat = np.ascontiguousarray(bias.reshape(1, COUT))
    return patches, wflat, bflat


_NC_CACHE = None


def _get_nc():
    global _NC_CACHE
    if _NC_CACHE is None:
        _NC_CACHE = _build_bass()
    return _NC_CACHE


def _run(inputs, trace=False):
    from concourse.bass_utils import run_bass_kernel_spmd

    patches, wflat, bflat = _host_prep(inputs)
    in_maps = []
    for core in range(NCORES):
        b, half = core // HALVES, core % HALVES
        shard = np.ascontiguousarray(
            patches[b, half * POS_PER_CORE : (half + 1) * POS_PER_CORE, :]
        )
        in_maps.append({"patches": shard, "w": wflat, "bias": bflat})

    nc = _get_nc()
    res = run_bass_kernel_spmd(nc, in_maps, core_ids=list(range(NCORES)), trace=trace)

    rows_per_half = POS_PER_CORE // WOUT  # 32
    out = np.empty((B, COUT, HOUT, WOUT), dtype=np.float32)
    for core in range(NCORES):
        b, half = core // HALVES, core % HALVES
        o = res.results[core]["out"]  # [POS_PER_CORE, COUT]
        out[b, :, half * rows_per_half : (half + 1) * rows_per_half, :] = o.T.reshape(
            COUT, rows_per_half, WOUT
        )
    return out, res


def kernel(**inputs) -> np.ndarray:
    out, _ = _run(inputs, trace=_TRACE)
    return out


# revision 21
# speedup vs baseline: 1.6198x; 1.0936x over previous
"""L-infinity distance "convolution" kernel for Trainium2 (8 NeuronCores).

Computes out[b, co, h, w] = max_acc |weights[co, acc] - patch[b, h, w, acc]| + bias[co]
where patches are 3x3 replicate-padded windows over x (4, 16, 64, 64),
acc = (c, kh, kw) ordered, accl = 16*9 = 144, cout = 64.

Sharding: 8 cores = 4 batches x 2 row-halves. Each core computes a
[2048 positions, 64 cout] shard. No collectives needed.

Device layout: partitions = 128 spatial positions per tile (16 tiles/core),
weights replicated across partitions (one broadcast DMA per 8-cout chunk).
Two compute paths, interleaved across the engines:
  - DVE path: one custom fused DVE instruction per (tile, cout): scan-max
    over |w - patch| plus bias, written through a step-0 output AP so the
    last scan element (the full 144-way reduction) lands in dist[:, co].
  - GPSIMD path (CFG["gps_count"] tiles): big tensor_tensor subtract on
    gpsimd (optionally + absmax pre-reduction levels), then a segmented
    tensor_reduce(max, abs) on DVE.
"""

import numpy as np

B, C, H, W = 4, 16, 64, 64
K = 3
COUT = 64
ACC = C * K * K  # 144
HOUT, WOUT = 64, 64
NPOS = HOUT * WOUT  # 4096
NCORES = 8
HALVES = 2
POS_PER_CORE = NPOS // HALVES  # 2048
P = 128  # partitions
NTILES = POS_PER_CORE // P  # 16
COG = 8  # cout chunk for weight broadcast tiles
NCHUNK = COUT // COG  # 8

# tuning knobs (A/B-tested via TimelineSim)
CFG = {
    "gps_count": 8,  # how many of the 16 tiles go to gpsimd
    "gps_tree": 0,  # levels of gpsimd absmax pre-reduction after the sub
    "mix_tiles": 4,  # squash tiles that donate their last cout-group to gpsimd
    "gps_whole_w": False,  # gps sub as one op reading a whole-weights tile
}

_TRACE = False

_OP_CACHE = None


def _get_op():
    """Register (once) the fused |a-b| scan-max (+bias) custom DVE op."""
    global _OP_CACHE
    if _OP_CACHE is not None:
        return _OP_CACHE
    from concourse.dve_spec import (
        Spec,
        Src0,
        Src1,
        C0,
        maxx,
        lower,
        AluOp,
        _has_src1,
        scan,
    )
    from concourse.dve_uop import DveOpSpec
    import concourse.dve_ops as dve_ops
    from concourse.dve_ops import DveOp

    def _ref(in0, in1, s0, s1, imm2):
        b = np.maximum.accumulate(np.abs(in0.astype(np.float32) - in1), axis=-1)
        return (b + s0).astype(np.float32)

    spec = Spec(
        body=scan(AluOp.MAX, maxx(Src0 - Src1, Src1 - Src0)) + C0, reference=_ref
    )
    name = "ABSDIFF_MAX_SCAN_B"
    if name not in dve_ops._SUB_OPCODE_FOR_NAME:
        row = max(dve_ops._SUB_OPCODE_FOR_NAME.values()) + 1
        assert row < 0x20
        dve_ops._SUB_OPCODE_FOR_NAME[name] = row
    row = dve_ops._SUB_OPCODE_FOR_NAME[name]
    shas = {}
    for ver in ("v3", "v4"):
        s = DveOpSpec(
            name=name, opcode=row, uops=lower(spec, ver=ver), rd1_en=_has_src1(spec)
        )
        shas[ver] = s.sha(ver)
    op = DveOp(name, spec, subdim=False, uops_sha=shas)
    if all(o.name != name for o in dve_ops.OPS):
        dve_ops.OPS.append(op)
        dve_ops.CUSTOM_DVE_SPECS[name] = spec
    _OP_CACHE = op
    return op


def _build_bass():
    import concourse.bass as bass
    import concourse.bacc as bacc
    import concourse.mybir as mybir
    import concourse.tile as tile
    from concourse.alu_op_type import AluOpType

    op = _get_op()

    nc = bacc.Bacc("TRN2", target_bir_lowering=False, debug=False, num_devices=NCORES)
    patches_d = nc.dram_tensor(
        "patches", [POS_PER_CORE, ACC], mybir.dt.float32, kind="ExternalInput"
    )
    w_d = nc.dram_tensor("w", [1, COUT * ACC], mybir.dt.float32, kind="ExternalInput")
    bias_d = nc.dram_tensor("bias", [1, COUT], mybir.dt.float32, kind="ExternalInput")
    out_d = nc.dram_tensor(
        "out", [POS_PER_CORE, COUT], mybir.dt.float32, kind="ExternalOutput"
    )

    # gpsimd tiles interleaved with DVE tiles; odd positions first so the
    # DVE starts on tile 0 immediately.
    kg = CFG["gps_count"]
    order = list(range(1, NTILES, 2)) + list(range(0, NTILES, 2))
    gps_tiles = set(order[:kg])
    tree_levels = CFG["gps_tree"]
    # squash tiles whose last cout-group is donated to gpsimd
    squash_order = [t for t in order if t not in gps_tiles]
    mix_tiles = set(squash_order[: CFG["mix_tiles"]])

    with tile.TileContext(nc) as tc:
        with (
            tc.tile_pool(name="consts", bufs=1) as consts,
            tc.tile_pool(name="work", bufs=4) as work,
            tc.tile_pool(name="dwork", bufs=2) as dwork,
            tc.tile_pool(name="outp", bufs=4) as outp,
        ):
            # weights replicated across partitions, in NCHUNK separate tiles
            # so early compute only waits on its own chunk's DMA
            wchunks = []
            for g in range(NCHUNK):
                wt = consts.tile([P, COG * ACC], mybir.dt.float32, tag=f"wch{g}")
                base = w_d[0:1, g * COG * ACC : (g + 1) * COG * ACC]
                src = bass.AP(
                    tensor=base.tensor, offset=base.offset, ap=[[0, P], [1, COG * ACC]]
                )
                nc.sync.dma_start(out=wt[:, :], in_=src)
                wchunks.append(wt)
            wbig = None
            if gps_tiles and CFG["gps_whole_w"]:
                wbig = consts.tile([P, COUT * ACC], mybir.dt.float32)
                base = w_d[0:1, :]
                src = bass.AP(
                    tensor=base.tensor, offset=base.offset, ap=[[0, P], [1, COUT * ACC]]
                )
                nc.sync.dma_start(out=wbig[:, :], in_=src)
            bias_rep = consts.tile([P, COUT], mybir.dt.float32)
            bbase = bias_d[0:1, :]
            bsrc = bass.AP(
                tensor=bbase.tensor, offset=bbase.offset, ap=[[0, P], [1, COUT]]
            )
            nc.sync.dma_start(out=bias_rep[:, :], in_=bsrc)

            pending = []  # gpsimd tiles awaiting their DVE reduce

            def flush_pending():
                while pending:
                    t0, red_in, inner = pending.pop(0)
                    dist = outp.tile([P, COUT], mybir.dt.float32, tag="dist")
                    r3 = red_in[:, :].rearrange("p (c a) -> p c a", a=inner)
                    nc.vector.tensor_reduce(
                        out=dist[:, :],
                        in_=r3,
                        axis=mybir.AxisListType.X,
                        op=AluOpType.max,
                        apply_absolute_value=True,
                    )
                    nc.vector.tensor_tensor(
                        out=dist[:, :],
                        in0=dist[:, :],
                        in1=bias_rep[:, :],
                        op=AluOpType.add,
                    )
                    nc.sync.dma_start(
                        out=out_d[t0 * P : (t0 + 1) * P, :], in_=dist[:, :]
                    )

            for t in range(NTILES):
                pt = work.tile([P, ACC], mybir.dt.float32, tag="pt")
                nc.sync.dma_start(out=pt[:, :], in_=patches_d[t * P : (t + 1) * P, :])
                if t in gps_tiles:
                    # chunked subtract: one gpsimd op per cout-group, reading
                    # its weight chunk + the patch broadcast
                    d_t = dwork.tile([P, COUT * ACC], mybir.dt.float32, tag="D")
                    if wbig is not None:
                        pt_b = pt[:, :].unsqueeze(1).broadcast_to([P, COUT, ACC])
                        nc.gpsimd.tensor_tensor(
                            out=d_t[:, :].rearrange("p (c a) -> p c a", a=ACC),
                            in0=wbig[:, :].rearrange("p (c a) -> p c a", a=ACC),
                            in1=pt_b,
                            op=AluOpType.subtract,
                        )
                    else:
                        pt_b = pt[:, :].unsqueeze(1).broadcast_to([P, COG, ACC])
                        for g in range(NCHUNK):
                            d3g = d_t[
                                :, g * COG * ACC : (g + 1) * COG * ACC
                            ].rearrange("p (c a) -> p c a", a=ACC)
                            w3g = wchunks[g][:, :].rearrange("p (c a) -> p c a", a=ACC)
                            nc.gpsimd.tensor_tensor(
                                out=d3g, in0=w3g, in1=pt_b, op=AluOpType.subtract
                            )
                    red_in, inner = d_t, ACC
                    if tree_levels >= 1:
                        t2 = dwork.tile(
                            [P, COUT * (ACC // 2)], mybir.dt.float32, tag="T2"
                        )
                        h = ACC // 2  # 72
                        a_v = d_t[:, :].rearrange("p (c a) -> p c a", a=ACC)
                        nc.gpsimd.tensor_tensor(
                            out=t2[:, :].rearrange("p (c a) -> p c a", a=h),
                            in0=a_v[:, :, 0:h],
                            in1=a_v[:, :, h:ACC],
                            op=AluOpType.abs_max,
                        )
                        red_in, inner = t2, h
                    if tree_levels >= 2:
                        t3 = dwork.tile(
                            [P, COUT * (ACC // 4)], mybir.dt.float32, tag="T3"
                        )
                        h2 = ACC // 4  # 36
                        a_v = red_in[:, :].rearrange("p (c a) -> p c a", a=inner)
                        nc.gpsimd.tensor_tensor(
                            out=t3[:, :].rearrange("p (c a) -> p c a", a=h2),
                            in0=a_v[:, :, 0:h2],
                            in1=a_v[:, :, h2:inner],
                            op=AluOpType.abs_max,
                        )
                        red_in, inner = t3, h2
                    pending.append((t, red_in, inner))
                else:
                    n_sq = COUT - COG if t in mix_tiles else COUT
                    dist = outp.tile([P, COUT], mybir.dt.float32, tag="dist")
                    dm = None
                    if t in mix_tiles:
                        dm = dwork.tile([P, COG * ACC], mybir.dt.float32, tag="Dm")
                        pt_b = pt[:, :].unsqueeze(1).broadcast_to([P, COG, ACC])
                        gl = NCHUNK - 1
                        w3g = wchunks[gl][:, :].rearrange("p (c a) -> p c a", a=ACC)
                        nc.gpsimd.tensor_tensor(
                            out=dm[:, :].rearrange("p (c a) -> p c a", a=ACC),
                            in0=w3g,
                            in1=pt_b,
                            op=AluOpType.subtract,
                        )
                    for co in range(n_sq):
                        g, j = co // COG, co % COG
                        w_slice = wchunks[g][:, j * ACC : (j + 1) * ACC]
                        d0 = dist[:, co : co + 1]
                        squash = bass.AP(
                            tensor=d0.tensor, offset=d0.offset, ap=[d0.ap[0], [0, ACC]]
                        )
                        nc.vector._custom_dve(
                            op,
                            out=squash,
                            in0=w_slice,
                            in1=pt[:, :],
                            s0=bias_rep[:, co : co + 1],
                        )
                    if dm is not None:
                        r3 = dm[:, :].rearrange("p (c a) -> p c a", a=ACC)
                        nc.vector.tensor_reduce(
                            out=dist[:, n_sq:COUT],
                            in_=r3,
                            axis=mybir.AxisListType.X,
                            op=AluOpType.max,
                            apply_absolute_value=True,
                        )
                        nc.vector.tensor_tensor(
                            out=dist[:, n_sq:COUT],
                            in0=dist[:, n_sq:COUT],
                            in1=bias_rep[:, n_sq:COUT],
                            op=AluOpType.add,
                        )
                    nc.sync.dma_start(
                        out=out_d[t * P : (t + 1) * P, :], in_=dist[:, :]
                    )
                    flush_pending()
            flush_pending()
    nc.compile()
    return nc


def _host_prep(inputs):
    x = np.asarray(inputs["x"], dtype=np.float32)
    weights = np.asarray(inputs["weights"], dtype=np.float32)
    bias = np.asarray(inputs["bias"], dtype=np.float32)
    assert x.shape == (B, C, H, W)
    assert weights.shape == (COUT, ACC)

    x_pad = np.pad(x, ((0, 0), (0, 0), (1, 1), (1, 1)), mode="edge")
    from numpy.lib.stride_tricks import sliding_window_view

    pw = sliding_window_view(x_pad, (K, K), axis=(2, 3))  # (B, C, HOUT, WOUT, K, K)
    patches = np.ascontiguousarray(pw.transpose(0, 2, 3, 1, 4, 5)).reshape(
        B, NPOS, ACC
    )
    wflat = np.ascontiguousarray(weights.reshape(1, COUT * ACC))
    bflat = np.ascontiguousarray(bias.reshape(1, COUT))
    return patches, wflat, bflat


_NC_CACHE = None


def _get_nc():
    global _NC_CACHE
    if _NC_CACHE is None:
        _NC_CACHE = _build_bass()
    return _NC_CACHE


def _run(inputs, trace=False):
    from concourse.bass_utils import run_bass_kernel_spmd

    patches, wflat, bflat = _host_prep(inputs)
    in_maps = []
    for core in range(NCORES):
        b, half = core // HALVES, core % HALVES
        shard = np.ascontiguousarray(
            patches[b, half * POS_PER_CORE : (half + 1) * POS_PER_CORE, :]
        )
        in_maps.append({"patches": shard, "w": wflat, "bias": bflat})

    nc = _get_nc()
    res = run_bass_kernel_spmd(nc, in_maps, core_ids=list(range(NCORES)), trace=trace)

    rows_per_half = POS_PER_CORE // WOUT  # 32
    out = np.empty((B, COUT, HOUT, WOUT), dtype=np.float32)
    for core in range(NCORES):
        b, half = core // HALVES, core % HALVES
        o = res.results[core]["out"]  # [POS_PER_CORE, COUT]
        out[b, :, half * rows_per_half : (half + 1) * rows_per_half, :] = o.T.reshape(
            COUT, rows_per_half, WOUT
        )
    return out, res


def kernel(**inputs) -> np.ndarray:
    out, _ = _run(inputs, trace=_TRACE)
    return out
